# revision 1
# baseline (speedup 1.0000x reference)
"""GumbelQuantizer forward on 8 Trainium2 NeuronCores.

Strategy (data-parallel over the bs*l token axis, per the sharding hint):
  - 32768 tokens are split into 8 shards of 4096 tokens; each core runs an
    identical Bass/Tile program on its shard. Weights + codebook replicated.
  - Per core:  hT = gelu(W1.T @ xT + b1)   (PE, f32r full-rate matmuls)
               logits = hT.T @ W2          (PE, token-major output)
               z = logits + (gumbels + b2) (DVE; b2 pre-folded into gumbels
                                            on host — mathematically identical)
               idx = argmax(z) per group   (DVE max/max_index)
               out = emb[idx]              (one batched indirect-DMA gather
                                            per 512-token chunk)
  - The straight-through estimator's forward value is hard one-hot up to
    ~1.2e-7, so the softmax itself is skipped and the output is the gathered
    codebook row (exact fp32).
  - x is transposed on host (the contraction dim must sit on partitions) and
    x/W1/W2 are pre-rounded to the f32r grid (11 explicit mantissa bits, RNE)
    so DMA can feed f32r matmul operands directly.
"""

import os
import sys

sys.path.insert(0, "/opt/trn_rl_repo")

import numpy as np

NCORES = 8
BS, L, DIM = 16, 2048, 512
NTOK = BS * L              # 32768 tokens total
TOK = NTOK // NCORES       # 4096 tokens per core
INNER = 1024
CODES = 320
G = 2
VD = 128                   # codebook row dim
CHUNK = 512                # tokens per pipeline chunk
NCHUNK = TOK // CHUNK      # 8
KT1 = DIM // 128           # 4  k-tiles for mm1
IT = INNER // 128          # 8  inner tiles
TT = CHUNK // 128          # 4  token sub-tiles per chunk

_CACHE = {}


def _round_f32r(a: np.ndarray) -> np.ndarray:
    """Round fp32 values to the f32r grid (drop 12 mantissa bits, RNE)."""
    u = np.ascontiguousarray(a, np.float32).view(np.uint32).copy()
    low = u & 0xFFF
    keep = u & np.uint32(0xFFFFF000)
    round_up = (low > 0x800) | ((low == 0x800) & (((u >> 12) & 1) == 1))
    keep = keep + (round_up.astype(np.uint32) << 12)
    return keep.view(np.float32)


def _build_nc():
    import concourse.bass as bass
    import concourse.tile as tile
    from concourse import bacc, mybir

    f32 = mybir.dt.float32
    f32r = mybir.dt.float32r
    u32 = mybir.dt.uint32
    ADD = mybir.AluOpType.add
    GELU = mybir.ActivationFunctionType.Gelu

    nc = bacc.Bacc("TRN2")
    xT = nc.dram_tensor("xT", [DIM, TOK], f32r, kind="ExternalInput")
    gum = nc.dram_tensor("gum", [TOK * G, CODES], f32, kind="ExternalInput")
    W1 = nc.dram_tensor("W1", [DIM, INNER], f32r, kind="ExternalInput")
    W2 = nc.dram_tensor("W2", [INNER, G * CODES], f32r, kind="ExternalInput")
    b1 = nc.dram_tensor("b1", [INNER], f32, kind="ExternalInput")
    emb = nc.dram_tensor("emb", [CODES, VD], f32, kind="ExternalInput")
    out = nc.dram_tensor("out", [TOK, G * VD], f32, kind="ExternalOutput")

    with tile.TileContext(nc) as tc:
        with (
            tc.tile_pool(name="consts", bufs=1) as consts,
            tc.tile_pool(name="xp", bufs=2) as xp,
            tc.tile_pool(name="hp", bufs=2) as hp,
            tc.tile_pool(name="gp", bufs=2) as gp,
            tc.tile_pool(name="zp", bufs=4) as zp,
            tc.tile_pool(name="mp", bufs=8) as mp,
            tc.tile_pool(name="op", bufs=2) as op,
            tc.tile_pool(name="ps1", bufs=2, space="PSUM") as ps1,
            tc.tile_pool(name="ps2", bufs=3, space="PSUM") as ps2,
        ):
            w1sb = consts.tile([128, KT1, INNER], f32r)
            nc.sync.dma_start(w1sb[:], W1.rearrange("(k p) i -> p k i", p=128))
            w2sb = consts.tile([128, IT, G * CODES], f32r)
            nc.sync.dma_start(w2sb[:], W2.rearrange("(k p) c -> p k c", p=128))
            b1sb = consts.tile([128, IT], f32)
            nc.sync.dma_start(b1sb[:], b1.rearrange("(i p) -> p i", p=128))

            xTr = xT.rearrange("(k p) t -> p k t", p=128)
            # gumbels: row 2*tok+g -> [chunk, part(token), t, g, code]
            gumr = gum.rearrange("(c t p g) x -> c p t g x",
                                 t=TT, p=128, g=G)
            outr = out.rearrange("(c t p) x -> c p t x", t=TT, p=128)

            for ch in range(NCHUNK):
                xsb = xp.tile([128, KT1, CHUNK], f32r)
                nc.sync.dma_start(xsb[:], xTr[:, :, ch * CHUNK:(ch + 1) * CHUNK])

                hsb = hp.tile([128, IT, CHUNK], f32r)
                for i in range(IT):
                    ph = ps1.tile([128, CHUNK], f32)
                    for k in range(KT1):
                        nc.tensor.matmul(
                            ph[:],
                            w1sb[:, k, i * 128:(i + 1) * 128],
                            xsb[:, k, :],
                            start=(k == 0),
                            stop=(k == KT1 - 1),
                        )
                    nc.scalar.activation(hsb[:, i, :], ph[:], GELU,
                                         bias=b1sb[:, i:i + 1])

                gsb = gp.tile([128, TT, G, CODES], f32)
                nc.scalar.dma_start(gsb[:], gumr[ch])

                osb = op.tile([128, TT, G * VD], f32)
                for t in range(TT):
                    pz = ps2.tile([128, G, 512], f32)
                    for k in range(IT):
                        for g2 in range(G):
                            nc.tensor.matmul(
                                pz[:, g2, 0:CODES],
                                hsb[:, k, t * 128:(t + 1) * 128],
                                w2sb[:, k, g2 * CODES:(g2 + 1) * CODES],
                                start=(k == 0),
                                stop=(k == IT - 1),
                            )
                    zsb = zp.tile([128, G, CODES], f32)
                    nc.vector.tensor_tensor(zsb[:], pz[:, :, 0:CODES],
                                            gsb[:, t], op=ADD)
                    for g2 in range(G):
                        m8 = mp.tile([128, 8], f32)
                        mi = mp.tile([128, 8], u32)
                        nc.vector.max(m8[:], zsb[:, g2, :])
                        nc.vector.max_index(mi[:], m8[:], zsb[:, g2, :])
                        nc.gpsimd.indirect_dma_start(
                            out=osb[:, t, g2 * VD:(g2 + 1) * VD],
                            out_offset=None,
                            in_=emb[:],
                            in_offset=bass.IndirectOffsetOnAxis(ap=mi[:, 0:1],
                                                                axis=0),
                        )
                nc.sync.dma_start(outr[ch], osb[:])

    nc.compile()
    return nc


def kernel(**inputs) -> np.ndarray:
    from concourse.bass_utils import run_bass_kernel_spmd

    x = np.asarray(inputs["x"], np.float32)
    gumbels = np.asarray(inputs["gumbels"], np.float32)
    W1 = np.asarray(inputs["W1"], np.float32)
    b1 = np.asarray(inputs["b1"], np.float32)
    W2 = np.asarray(inputs["W2"], np.float32)
    b2 = np.asarray(inputs["b2"], np.float32)
    emb = np.asarray(inputs["emb"], np.float32)

    if "nc" not in _CACHE:
        _CACHE["nc"] = _build_nc()
    nc = _CACHE["nc"]

    xt = x.reshape(NTOK, DIM)
    W1r = _round_f32r(W1)
    W2r = _round_f32r(W2)
    # fold b2 into the gumbel noise: z = logits + b2 + gumbels
    gumb = gumbels.reshape(NTOK, G, CODES) + b2.reshape(G, CODES)
    gumb = gumb.reshape(NTOK * G, CODES)

    in_maps = []
    for c in range(NCORES):
        xT_c = _round_f32r(np.ascontiguousarray(xt[c * TOK:(c + 1) * TOK, :].T))
        in_maps.append({
            "xT": xT_c,
            "gum": np.ascontiguousarray(gumb[c * TOK * G:(c + 1) * TOK * G]),
            "W1": W1r,
            "W2": W2r,
            "b1": b1,
            "emb": emb,
        })

    trace = bool(int(os.environ.get("KERNEL_TRACE", "0")))
    res = run_bass_kernel_spmd(nc, in_maps, core_ids=list(range(NCORES)),
                               trace=trace)
    _CACHE["last_result"] = res
    outs = [res.results[c]["out"] for c in range(NCORES)]
    return np.concatenate(outs, axis=0).reshape(BS, L, G * VD)



# revision 2
# speedup vs baseline: 1.0608x; 1.0608x over previous
"""GumbelQuantizer forward on 8 Trainium2 NeuronCores.

Strategy (data-parallel over the bs*l token axis, per the sharding hint):
  - 32768 tokens are split into 8 shards of 4096 tokens; each core runs an
    identical Bass/Tile program on its shard. Weights + codebook replicated.
  - Per core:  hT = gelu(W1.T @ xT + b1)   (PE, f32r full-rate matmuls)
               logits = hT.T @ W2          (PE, token-major output)
               z = logits + (gumbels + b2) (DVE; b2 pre-folded into gumbels
                                            on host — mathematically identical)
               idx = argmax(z) per group   (DVE max/max_index)
               out = emb[idx]              (indirect-DMA gather per 128-token
                                            subtile and group)
  - The straight-through estimator's forward value is hard one-hot up to
    ~1.2e-7, so the softmax itself is skipped and the output is the gathered
    codebook row (exact fp32).
  - All DRAM operands are pre-swizzled on host into [128, ...] partition-major
    layouts so every DMA is 128 large contiguous descriptors (fast HWDGE
    descriptor generation, no ring backpressure).
  - W1 is sliced 4-ways / W2 2-ways so the first matmuls start as soon as the
    first ~512KB of weights land instead of after all weight DMA completes.
"""

import os
import sys

sys.path.insert(0, "/opt/trn_rl_repo")

import numpy as np

NCORES = 8
BS, L, DIM = 16, 2048, 512
NTOK = BS * L              # 32768 tokens total
TOK = NTOK // NCORES       # 4096 tokens per core
INNER = 1024
CODES = 320
G = 2
VD = 128                   # codebook row dim
CHUNK = 512                # tokens per pipeline chunk
NCHUNK = TOK // CHUNK      # 8
KT1 = DIM // 128           # 4  k-tiles for mm1
IT = INNER // 128          # 8  inner tiles
TT = CHUNK // 128          # 4  token sub-tiles per chunk
W1S = 4                    # W1 DMA slices (over inner blocks)
W1B = INNER // W1S // 128  # 2  128-wide inner blocks per slice
W2S = 2                    # W2 DMA slices (over k)
W2K = IT // W2S            # 4  k-tiles per W2 slice

_CACHE = {}


def _round_f32r(a: np.ndarray) -> np.ndarray:
    """Round fp32 values to the f32r grid (drop 12 mantissa bits, RNE)."""
    u = np.ascontiguousarray(a, np.float32).view(np.uint32).copy()
    low = u & 0xFFF
    keep = u & np.uint32(0xFFFFF000)
    round_up = (low > 0x800) | ((low == 0x800) & (((u >> 12) & 1) == 1))
    keep = keep + (round_up.astype(np.uint32) << 12)
    return keep.view(np.float32)


def _build_nc():
    import concourse.bass as bass
    import concourse.tile as tile
    from concourse import bacc, mybir

    f32 = mybir.dt.float32
    f32r = mybir.dt.float32r
    u32 = mybir.dt.uint32
    ADD = mybir.AluOpType.add
    GELU = mybir.ActivationFunctionType.Gelu

    nc = bacc.Bacc("TRN2")
    # Host-swizzled layouts: partition dim first, per-chunk slices contiguous.
    xA = nc.dram_tensor("xA", [128, NCHUNK, KT1, CHUNK], f32r,
                        kind="ExternalInput")
    gA = nc.dram_tensor("gA", [128, NCHUNK, TT, G, CODES], f32,
                        kind="ExternalInput")
    w1A = nc.dram_tensor("w1A", [128, W1S, KT1, W1B * 128], f32r,
                         kind="ExternalInput")
    w2A = nc.dram_tensor("w2A", [128, W2S, W2K, G * CODES], f32r,
                         kind="ExternalInput")
    b1A = nc.dram_tensor("b1A", [128, IT], f32, kind="ExternalInput")
    emb = nc.dram_tensor("emb", [CODES, VD], f32, kind="ExternalInput")
    outA = nc.dram_tensor("outA", [128, NCHUNK, TT, G * VD], f32,
                          kind="ExternalOutput")

    with tile.TileContext(nc) as tc:
        with (
            tc.tile_pool(name="consts", bufs=1) as consts,
            tc.tile_pool(name="xp", bufs=3) as xp,
            tc.tile_pool(name="hp", bufs=2) as hp,
            tc.tile_pool(name="gp", bufs=3) as gp,
            tc.tile_pool(name="zp", bufs=4) as zp,
            tc.tile_pool(name="mp", bufs=8) as mp,
            tc.tile_pool(name="op", bufs=2) as op,
            tc.tile_pool(name="ps1", bufs=2, space="PSUM") as ps1,
            tc.tile_pool(name="ps2", bufs=3, space="PSUM") as ps2,
        ):
            # --- weight/bias loads: scalar HWDGE queue, sliced for early start
            w1s = []
            for s in range(W1S):
                w = consts.tile([128, KT1, W1B * 128], f32r, tag=f"w1_{s}")
                nc.scalar.dma_start(w[:], w1A[:, s])
                w1s.append(w)
            b1sb = consts.tile([128, IT], f32)
            nc.scalar.dma_start(b1sb[:], b1A[:])
            w2s = []
            for s in range(W2S):
                w = consts.tile([128, W2K, G * CODES], f32r, tag=f"w2_{s}")
                nc.scalar.dma_start(w[:], w2A[:, s])
                w2s.append(w)

            for ch in range(NCHUNK):
                # inputs for this chunk: x on sync queue, gumbels on scalar
                xsb = xp.tile([128, KT1, CHUNK], f32r)
                nc.sync.dma_start(xsb[:], xA[:, ch])
                gsb = gp.tile([128, TT, G, CODES], f32)
                nc.scalar.dma_start(gsb[:], gA[:, ch])

                # mm1 + gelu: h[i] = gelu(W1[:, i].T @ x + b1[i])
                hs = []
                for i in range(IT):
                    s, j = divmod(i, W1B)
                    ph = ps1.tile([128, CHUNK], f32)
                    for k in range(KT1):
                        nc.tensor.matmul(
                            ph[:],
                            w1s[s][:, k, j * 128:(j + 1) * 128],
                            xsb[:, k, :],
                            start=(k == 0),
                            stop=(k == KT1 - 1),
                        )
                    h = hp.tile([128, CHUNK], f32r, tag=f"h{i}")
                    nc.scalar.activation(h[:], ph[:], GELU,
                                         bias=b1sb[:, i:i + 1])
                    hs.append(h)

                osb = op.tile([128, TT, G * VD], f32)
                for t in range(TT):
                    pz = ps2.tile([128, G, 512], f32)
                    for k in range(IT):
                        s, kl = divmod(k, W2K)
                        for g2 in range(G):
                            nc.tensor.matmul(
                                pz[:, g2, 0:CODES],
                                hs[k][:, t * 128:(t + 1) * 128],
                                w2s[s][:, kl, g2 * CODES:(g2 + 1) * CODES],
                                start=(k == 0),
                                stop=(k == IT - 1),
                            )
                    zsb = zp.tile([128, G, CODES], f32)
                    nc.vector.tensor_tensor(zsb[:], pz[:, :, 0:CODES],
                                            gsb[:, t], op=ADD)
                    for g2 in range(G):
                        m8 = mp.tile([128, 8], f32)
                        mi = mp.tile([128, 8], u32)
                        nc.vector.max(m8[:], zsb[:, g2, :])
                        nc.vector.max_index(mi[:], m8[:], zsb[:, g2, :])
                        nc.gpsimd.indirect_dma_start(
                            out=osb[:, t, g2 * VD:(g2 + 1) * VD],
                            out_offset=None,
                            in_=emb[:],
                            in_offset=bass.IndirectOffsetOnAxis(ap=mi[:, 0:1],
                                                                axis=0),
                        )
                nc.sync.dma_start(outA[:, ch], osb[:])

    nc.compile()
    return nc


def kernel(**inputs) -> np.ndarray:
    from concourse.bass_utils import run_bass_kernel_spmd

    x = np.asarray(inputs["x"], np.float32)
    gumbels = np.asarray(inputs["gumbels"], np.float32)
    W1 = np.asarray(inputs["W1"], np.float32)
    b1 = np.asarray(inputs["b1"], np.float32)
    W2 = np.asarray(inputs["W2"], np.float32)
    b2 = np.asarray(inputs["b2"], np.float32)
    emb = np.asarray(inputs["emb"], np.float32)

    if "nc" not in _CACHE:
        _CACHE["nc"] = _build_nc()
    nc = _CACHE["nc"]

    xt = x.reshape(NTOK, DIM)
    # weight swizzles: [128, slice, k, cols] partition-major contiguous
    W1r = _round_f32r(W1)
    w1A = np.ascontiguousarray(
        W1r.reshape(KT1, 128, W1S, W1B * 128).transpose(1, 2, 0, 3))
    W2r = _round_f32r(W2)
    w2A = np.ascontiguousarray(
        W2r.reshape(W2S, W2K, 128, G * CODES).transpose(2, 0, 1, 3))
    b1A = np.ascontiguousarray(b1.reshape(IT, 128).T)
    # fold b2 into the gumbel noise: z = logits + b2 + gumbels
    gumb = gumbels.reshape(NTOK, G, CODES) + b2.reshape(G, CODES)

    in_maps = []
    for c in range(NCORES):
        xs = _round_f32r(xt[c * TOK:(c + 1) * TOK])
        xA = np.ascontiguousarray(
            xs.reshape(NCHUNK, CHUNK, KT1, 128).transpose(3, 0, 2, 1))
        gs = gumb[c * TOK:(c + 1) * TOK]
        gA = np.ascontiguousarray(
            gs.reshape(NCHUNK, TT, 128, G, CODES).transpose(2, 0, 1, 3, 4))
        in_maps.append({
            "xA": xA,
            "gA": gA,
            "w1A": w1A,
            "w2A": w2A,
            "b1A": b1A,
            "emb": emb,
        })

    trace = bool(int(os.environ.get("KERNEL_TRACE", "0")))
    res = run_bass_kernel_spmd(nc, in_maps, core_ids=list(range(NCORES)),
                               trace=trace)
    _CACHE["last_result"] = res
    outs = []
    for c in range(NCORES):
        o = res.results[c]["outA"]  # [128, NCHUNK, TT, 256]
        outs.append(o.transpose(1, 2, 0, 3).reshape(TOK, G * VD))
    return np.concatenate(outs, axis=0).reshape(BS, L, G * VD)
